# revision 1
# baseline (speedup 1.0000x reference)
"""Trainium2 Bass kernel for nn_Cross_Attention_55671366091237.

Reference computation (B=4, N=2048, dim=512, H=8, dh=64):
    oq  = x @ W_fc + b_fc            # [B,N,64], modulates Q (bcast over heads)
    okv = y @ W_fc + b_fc            # [B,N,64], modulates K and V
    q,k,v = split(x @ W_qkv)         # each [B,N,512] -> heads [B,H,N,64]
    attn  = softmax(q*oq @ (k*okv)^T * dh^-0.5)
    out   = (attn @ (v*okv)) @ W_out + b_out

Sharding: 8 cores = 4 batches x 2 head-groups (4 heads each). Weights are
sliced per head-group host-side; x/y are passed pre-transposed ([dim, N])
so the contraction dim lands on SBUF partitions. Each core computes a
partial output projection over its 4 heads; the host sums the two partials
per batch and adds b_out.

On-chip layout: everything transposed. Projections produce Q^T/K^T [dh, N]
per head-pair (two heads stacked on 128 partitions) and V in natural [N, dh]
layout with a ones-column appended, so the attention V-matmul also produces
the softmax denominator (row 64 of the PSUM accumulator). S^T = K^T.T @ Q^T
tiles land in PSUM, one ACT Exp instruction (scale=1/8 fused) moves them to
SBUF, and fp32r matmuls (full-rate fp32) accumulate attn@V over k-tiles.
Normalization is a DVE reciprocal + rank-1 ones-outer-product broadcast.
"""

import numpy as np

B, N, DIM = 4, 2048, 512
HEADS, DH = 8, 64
N_CORES = 8
SCALE = DH ** -0.5  # 0.125

_RUNNER_CACHE = {}


# --------------------------------------------------------------------------
# Bass module
# --------------------------------------------------------------------------

def _build_nc(loop_n: int = 1):
    import concourse.mybir as mybir
    from concourse import bacc
    from concourse.tile import TileContext
    from concourse.masks import make_identity

    fp32 = mybir.dt.float32
    f32r = mybir.dt.float32r  # fp32 data at full matmul rate (producers round)
    Exp = mybir.ActivationFunctionType.Exp

    nc = bacc.Bacc("TRN2", target_bir_lowering=False, debug=False)

    xT = nc.dram_tensor("xT", [DIM, N], fp32, kind="ExternalInput")
    yT = nc.dram_tensor("yT", [DIM, N], fp32, kind="ExternalInput")
    wq_d = nc.dram_tensor("wq", [DIM, 256], fp32, kind="ExternalInput")
    wk_d = nc.dram_tensor("wk", [DIM, 256], fp32, kind="ExternalInput")
    wv_d = nc.dram_tensor("wv", [DIM, 256], fp32, kind="ExternalInput")
    wfc_d = nc.dram_tensor("wfc", [DIM, DH], fp32, kind="ExternalInput")
    bfc_d = nc.dram_tensor("bfc", [DH, 1], fp32, kind="ExternalInput")
    wo_d = nc.dram_tensor("wo", [256, DIM], fp32, kind="ExternalInput")
    out_d = nc.dram_tensor("out", [N, DIM], fp32, kind="ExternalOutput")

    NT = N // 128   # 16 n-tiles of 128
    NS = N // 512   # 4  n-slices of 512
    DT = DIM // 128  # 4 contraction tiles

    with TileContext(nc) as tc:
        import contextlib
        with contextlib.ExitStack() as ctx:
            const = ctx.enter_context(tc.tile_pool(name="const", bufs=1))
            xtp = ctx.enter_context(tc.tile_pool(name="xtp", bufs=DT))
            ytp = ctx.enter_context(tc.tile_pool(name="ytp", bufs=4))
            big = ctx.enter_context(tc.tile_pool(name="big", bufs=2))
            onep = ctx.enter_context(tc.tile_pool(name="onep", bufs=1))
            ep = ctx.enter_context(tc.tile_pool(name="ep", bufs=4))
            accsp = ctx.enter_context(tc.tile_pool(name="accsp", bufs=4))
            rp = ctx.enter_context(tc.tile_pool(name="rp", bufs=4))
            outp = ctx.enter_context(tc.tile_pool(name="outp", bufs=4))
            # PSUM: mixps 2 banks + sps 2x[128,1024]=4 banks + accps 2 = 8
            mixps = ctx.enter_context(
                tc.tile_pool(name="mixps", bufs=2, space="PSUM"))
            sps = ctx.enter_context(
                tc.tile_pool(name="sps", bufs=2, space="PSUM"))
            accps = ctx.enter_context(
                tc.tile_pool(name="accps", bufs=2, space="PSUM"))

            def body(_i=None):
                # ---- constants / weights -------------------------------
                wq = const.tile([128, DT, 256], f32r, tag="wq")
                nc.sync.dma_start(wq[:, :, :],
                                  wq_d.bitcast(f32r).rearrange("(t p) f -> p t f", p=128))
                wk = const.tile([128, DT, 256], f32r, tag="wk")
                nc.sync.dma_start(wk[:, :, :],
                                  wk_d.bitcast(f32r).rearrange("(t p) f -> p t f", p=128))
                wv = const.tile([128, DT, 256], f32r, tag="wv")
                nc.sync.dma_start(wv[:, :, :],
                                  wv_d.bitcast(f32r).rearrange("(t p) f -> p t f", p=128))
                # W_fc duplicated along free dim: one matmul yields the
                # modulation row-block for both heads of a pair.
                wfc2 = const.tile([128, DT, 128], f32r, tag="wfc2")
                wfc_r = wfc_d.bitcast(f32r).rearrange("(t p) f -> p t f", p=128)
                nc.sync.dma_start(wfc2[:, :, 0:DH], wfc_r)
                nc.sync.dma_start(wfc2[:, :, DH:128], wfc_r)
                bfc2 = const.tile([128, 1], fp32, tag="bfc2")
                nc.sync.dma_start(bfc2[0:DH, :], bfc_d[:, :])
                nc.sync.dma_start(bfc2[DH:128, :], bfc_d[:, :])
                wo = const.tile([128, 2, DIM], f32r, tag="wo")
                nc.sync.dma_start(wo[:, :, :],
                                  wo_d.bitcast(f32r).rearrange("(t p) f -> p t f", p=128))
                ident = const.tile([128, 128], fp32, tag="ident")
                make_identity(nc, ident[:, :])
                ones1 = const.tile([128, 1], fp32, tag="ones1")
                nc.gpsimd.memset(ones1[:, :], 1.0)
                ones_row = const.tile([1, DH], f32r, tag="ones_row")
                nc.vector.tensor_copy(ones_row[:, :],
                                      ones1[0:1, :].broadcast_to((1, DH)))

                xt = []
                for t in range(DT):
                    xtile = xtp.tile([128, N], f32r, tag="xt")
                    nc.sync.dma_start(xtile[:, :], xT.bitcast(f32r)[t * 128:(t + 1) * 128, :])
                    xt.append(xtile)

                # ---- oq^T / okv^T (both duplicated to 128 rows) --------
                oqT2 = onep.tile([128, N], fp32, tag="oqT2")
                okvT2 = onep.tile([128, N], fp32, tag="okvT2")
                for ns in range(NS):
                    sl = slice(ns * 512, (ns + 1) * 512)
                    ps = mixps.tile([128, 512], fp32, tag="ps")
                    for t in range(DT):
                        ytile = ytp.tile([128, 512], f32r, tag="yt")
                        nc.sync.dma_start(
                            ytile[:, :], yT.bitcast(f32r)[t * 128:(t + 1) * 128, sl])
                        nc.tensor.matmul(ps[:, :], wfc2[:, t, :],
                                         ytile[:, :],
                                         start=(t == 0), stop=(t == DT - 1))
                    nc.vector.tensor_scalar_add(okvT2[:, sl], ps[:, :],
                                                bfc2[:, :])

                for ns in range(NS):
                    sl = slice(ns * 512, (ns + 1) * 512)
                    ps = mixps.tile([128, 512], fp32, tag="ps")
                    for t in range(DT):
                        nc.tensor.matmul(ps[:, :], wfc2[:, t, :],
                                         xt[t][:, sl],
                                         start=(t == 0), stop=(t == DT - 1))
                    nc.vector.tensor_scalar_add(oqT2[:, sl], ps[:, :],
                                                bfc2[:, :])

                # okv in natural [N, dh] layout (for V modulation): PE
                # transpose of okv^T 128-column blocks.
                okvn = onep.tile([128, NT, DH], fp32, tag="okvn")
                for g in range(NT // 8):
                    tps = mixps.tile([128, 512], fp32, tag="ps")
                    for j in range(8):
                        nt = g * 8 + j
                        nc.tensor.transpose(
                            tps[:, j * DH:(j + 1) * DH],
                            okvT2[0:DH, nt * 128:(nt + 1) * 128],
                            ident[0:DH, 0:DH])
                    nc.vector.tensor_copy(okvn[:, g * 8:(g + 1) * 8, :],
                                          tps[:, :].rearrange(
                                              "p (n c) -> p n c", n=8))

                # ---- V projection for all 4 heads (natural layout,
                # ones column per head for the softmax denominator) ------
                v4 = big.tile([128, NT, 260], f32r, tag="v4")
                ones_b = ones1[:, :].unsqueeze(1).broadcast_to((128, NT, 1))
                v4h = v4[:, :, :].rearrange("p n (h c) -> p n h c", h=4)
                nc.vector.tensor_copy(v4h[:, :, :, DH:DH + 1],
                                      ones_b.unsqueeze(2).broadcast_to(
                                          (128, NT, 4, 1)))
                for nt in range(0, NT, 2):
                    psv = mixps.tile([128, 512], fp32, tag="ps")
                    for half in range(2):
                        for t in range(DT):
                            nc.tensor.matmul(
                                psv[:, half * 256:half * 256 + 256],
                                xt[t][:, (nt + half) * 128:
                                       (nt + half + 1) * 128],
                                wv[:, t, :],
                                start=(t == 0), stop=(t == DT - 1))
                    okb = okvn[:, nt:nt + 2, :].unsqueeze(2).broadcast_to(
                        (128, 2, 4, DH))
                    nc.vector.tensor_mul(
                        v4[:, nt:nt + 2, :].rearrange(
                            "p n (h c) -> p n h c", h=4)[:, :, :, 0:DH],
                        psv[:, :].rearrange("p (n h c) -> p n h c", n=2, h=4),
                        okb)

                def qk_proj(p, ns_list, qmod, kmod):
                    pf = slice(p * 128, (p + 1) * 128)
                    for ns in ns_list:
                        sl = slice(ns * 512, (ns + 1) * 512)
                        psq = mixps.tile([128, 512], fp32, tag="ps")
                        for t in range(DT):
                            nc.tensor.matmul(psq[:, :], wq[:, t, pf],
                                             xt[t][:, sl],
                                             start=(t == 0),
                                             stop=(t == DT - 1))
                        nc.vector.tensor_mul(qmod[:, sl], psq[:, :],
                                             oqT2[:, sl])
                        psk = mixps.tile([128, 512], fp32, tag="ps")
                        for t in range(DT):
                            nc.tensor.matmul(psk[:, :], wk[:, t, pf],
                                             xt[t][:, sl],
                                             start=(t == 0),
                                             stop=(t == DT - 1))
                        nc.vector.tensor_mul(kmod[:, sl], psk[:, :],
                                             okvT2[:, sl])

                def attn_qt(p, qt, qmod, kmod, ot):
                    qsl = slice(qt * 512, (qt + 1) * 512)
                    acc0 = accps.tile([65, 512], fp32, tag="acc")
                    acc1 = accps.tile([65, 512], fp32, tag="acc")
                    for kt in range(NT):
                        ksl = slice(kt * 128, (kt + 1) * 128)
                        sp = sps.tile([128, 1024], fp32, tag="s")
                        nc.tensor.matmul(sp[:, 0:512],
                                         kmod[0:DH, ksl],
                                         qmod[0:DH, qsl],
                                         start=True, stop=True)
                        nc.tensor.matmul(sp[:, 512:1024],
                                         kmod[DH:128, ksl],
                                         qmod[DH:128, qsl],
                                         start=True, stop=True)
                        e = ep.tile([128, 1024], f32r, tag="e")
                        nc.scalar.activation(e[:, :], sp[:, :], Exp,
                                             scale=float(SCALE))
                        nc.tensor.matmul(acc0[:, :],
                                         v4[:, kt, p * 130:p * 130 + 65],
                                         e[:, 0:512],
                                         start=(kt == 0),
                                         stop=(kt == NT - 1))
                        nc.tensor.matmul(acc1[:, :],
                                         v4[:, kt, p * 130 + 65:p * 130 + 130],
                                         e[:, 512:1024],
                                         start=(kt == 0),
                                         stop=(kt == NT - 1))
                    for h, acc in ((0, acc0), (1, acc1)):
                        accS = accsp.tile([65, 512], fp32, tag="accS")
                        nc.vector.tensor_copy(accS[:, :], acc[:, :])
                        rec = rp.tile([1, 512], f32r, tag="rec")
                        with nc.allow_low_precision(
                                reason="f32r-typed fp32 reciprocal row"):
                            nc.vector.reciprocal(rec[:, :], accS[64:65, :])
                        bc = mixps.tile([128, 512], fp32, tag="ps")
                        nc.tensor.matmul(bc[0:DH, :], ones_row[:, :],
                                         rec[:, :], start=True, stop=True)
                        nc.vector.tensor_mul(ot[h * DH:(h + 1) * DH, qsl],
                                             accS[0:DH, :], bc[0:DH, :])

                qmod0 = big.tile([128, N], f32r, tag="qmod")
                kmod0 = big.tile([128, N], f32r, tag="kmod")
                qk_proj(0, range(NS), qmod0, kmod0)
                qmod1 = big.tile([128, N], f32r, tag="qmod")
                kmod1 = big.tile([128, N], f32r, tag="kmod")
                ot0 = big.tile([128, N], f32r, tag="ot")
                ot1 = big.tile([128, N], f32r, tag="ot")
                ots = [ot0, ot1]
                # pair-0 attention, with pair-1 QK projection chunks
                # interleaved into the PE idle gaps of the ACT-bound loop
                for qt in range(NS):
                    attn_qt(0, qt, qmod0, kmod0, ot0)
                    if qt < 2:
                        qk_proj(1, range(2 * qt, 2 * qt + 2), qmod1, kmod1)
                def outproj_nt(nt):
                    nsl = slice(nt * 128, (nt + 1) * 128)
                    pso = mixps.tile([128, 512], fp32, tag="ps")
                    nc.tensor.matmul(pso[:, :], ots[0][:, nsl],
                                     wo[:, 0, :], start=True, stop=False)
                    nc.tensor.matmul(pso[:, :], ots[1][:, nsl],
                                     wo[:, 1, :], start=False, stop=True)
                    ob = outp.tile([128, 512], fp32, tag="ob")
                    nc.vector.tensor_copy(ob[:, :], pso[:, :])
                    nc.sync.dma_start(out_d[nsl, :], ob[:, :])

                for qt in range(NS):
                    attn_qt(1, qt, qmod1, kmod1, ot1)
                    for nt in range(4 * qt, 4 * qt + 4):
                        outproj_nt(nt)

            if loop_n > 1:
                with tc.For_i(0, loop_n, 1) as _i:
                    body(_i)
            else:
                body()

    nc.compile()
    return nc


# --------------------------------------------------------------------------
# PJRT SPMD runner (axon path) — keeps the jitted callable for reuse
# --------------------------------------------------------------------------

class _SpmdRunner:
    def __init__(self, nc, n_cores):
        import jax
        from jax.sharding import Mesh, PartitionSpec, NamedSharding
        from jax.experimental.shard_map import shard_map
        import concourse.mybir as mybir
        from concourse import bass2jax
        from concourse.bass2jax import _bass_exec_p, install_neuronx_cc_hook

        install_neuronx_cc_hook()
        self.jax = jax
        self.nc = nc
        self.n_cores = n_cores
        pname = nc.partition_id_tensor.name if nc.partition_id_tensor else None
        in_names, out_names, out_avals, zero_shapes = [], [], [], []
        for alloc in nc.m.functions[0].allocations:
            if not isinstance(alloc, mybir.MemoryLocationSet):
                continue
            name = alloc.memorylocations[0].name
            if alloc.kind == "ExternalInput":
                if name != pname:
                    in_names.append(name)
            elif alloc.kind == "ExternalOutput":
                out_names.append(name)
                shape = tuple(alloc.tensor_shape)
                dtype = mybir.dt.np(alloc.dtype)
                out_avals.append(jax.core.ShapedArray(shape, dtype))
                zero_shapes.append((shape, dtype))
        self.n_params = len(in_names)
        self.in_names = list(in_names)
        self.out_names = out_names
        self.out_avals = out_avals
        all_names = in_names + out_names
        if pname is not None:
            all_names.append(pname)

        def _body(*args):
            operands = list(args)
            if pname is not None:
                operands.append(bass2jax.partition_id_tensor())
            return tuple(_bass_exec_p.bind(
                *operands, out_avals=tuple(out_avals),
                in_names=tuple(all_names), out_names=tuple(out_names),
                lowering_input_output_aliases=(),
                sim_require_finite=True, sim_require_nnan=True, nc=nc))

        devices = jax.devices()[:n_cores]
        self.mesh = Mesh(np.asarray(devices), ("core",))
        n_outs = len(out_avals)
        in_specs = (PartitionSpec("core"),) * (self.n_params + n_outs)
        out_specs = (PartitionSpec("core"),) * n_outs
        donate = tuple(range(self.n_params, self.n_params + n_outs))
        self.sharding = NamedSharding(self.mesh, PartitionSpec("core"))
        self.sharded = jax.jit(
            shard_map(_body, mesh=self.mesh, in_specs=in_specs,
                      out_specs=out_specs, check_rep=False),
            donate_argnums=donate, keep_unused=True)
        zs = [(n_cores * s[0], *s[1:]) for s, _ in zero_shapes]
        zd = [d for _, d in zero_shapes]
        self._mkzeros = jax.jit(
            lambda: tuple(jax.numpy.zeros(s, d) for s, d in zip(zs, zd)),
            out_shardings=tuple(self.sharding for _ in zs))

    def put_inputs(self, in_maps):
        concat = [np.concatenate(
            [np.ascontiguousarray(in_maps[c][n]) for c in range(self.n_cores)],
            axis=0) for n in self.in_names]
        return [self.jax.device_put(a, self.sharding) for a in concat]

    def run(self, in_dev):
        outs = self.sharded(*in_dev, *self._mkzeros())
        self.jax.block_until_ready(outs)
        return outs

    def results(self, outs):
        res = []
        for c in range(self.n_cores):
            d = {}
            for i, name in enumerate(self.out_names):
                full = np.asarray(outs[i])
                d[name] = full.reshape(self.n_cores,
                                       *self.out_avals[i].shape)[c]
            res.append(d)
        return res


def _get_runner(loop_n: int = 1):
    if loop_n not in _RUNNER_CACHE:
        nc = _build_nc(loop_n)
        _RUNNER_CACHE[loop_n] = _SpmdRunner(nc, N_CORES)
    return _RUNNER_CACHE[loop_n]


# --------------------------------------------------------------------------
# host-side shard / gather
# --------------------------------------------------------------------------

def _shard_inputs(x, y, W_qkv, W_fc, b_fc, W_out):
    in_maps = []
    for c in range(N_CORES):
        b, g = c // 2, c % 2
        hs = slice(g * 256, (g + 1) * 256)
        in_maps.append({
            "xT": np.ascontiguousarray(np.asarray(x[b]).T),
            "yT": np.ascontiguousarray(np.asarray(y[b]).T),
            "wq": np.ascontiguousarray(np.asarray(W_qkv)[:, hs]),
            "wk": np.ascontiguousarray(np.asarray(W_qkv)[:, 512:][:, hs]),
            "wv": np.ascontiguousarray(np.asarray(W_qkv)[:, 1024:][:, hs]),
            "wfc": np.ascontiguousarray(np.asarray(W_fc)),
            "bfc": np.ascontiguousarray(np.asarray(b_fc).reshape(DH, 1)),
            "wo": np.ascontiguousarray(np.asarray(W_out)[hs, :]),
        })
    return in_maps


def kernel(x, y, W_qkv, W_fc, b_fc, W_out, b_out):
    runner = _get_runner(1)
    in_maps = _shard_inputs(x, y, W_qkv, W_fc, b_fc, W_out)
    in_dev = runner.put_inputs(in_maps)
    res = runner.results(runner.run(in_dev))
    b_out = np.asarray(b_out, dtype=np.float32)
    out = np.empty((B, N, DIM), dtype=np.float32)
    for b in range(B):
        out[b] = res[2 * b]["out"] + res[2 * b + 1]["out"] + b_out
    return out



# revision 4
# speedup vs baseline: 1.1375x; 1.1375x over previous
"""Trainium2 Bass kernel for nn_Cross_Attention_55671366091237.

Reference computation (B=4, N=2048, dim=512, H=8, dh=64):
    oq  = x @ W_fc + b_fc            # [B,N,64], modulates Q (bcast over heads)
    okv = y @ W_fc + b_fc            # [B,N,64], modulates K and V
    q,k,v = split(x @ W_qkv)         # each [B,N,512] -> heads [B,H,N,64]
    attn  = softmax(q*oq @ (k*okv)^T * dh^-0.5)
    out   = (attn @ (v*okv)) @ W_out + b_out

Sharding: 8 cores = 4 batches x 2 head-groups (4 heads each). Weights are
sliced per head-group host-side; x/y are passed pre-transposed ([dim, N])
and pre-cast to bf16 so the contraction dim lands on SBUF partitions and
input DMA bytes are halved. Each core computes a partial output projection
over its 4 heads; the host sums the two partials per batch and adds b_out.

On-chip layout: everything transposed. Projections produce Q^T/K^T [dh, N]
per head-pair (two heads stacked on 128 partitions) and V in natural [N, dh]
layout with a ones-column appended, so the attention V-matmul also produces
the softmax denominator (row 64 of the PSUM accumulator). S^T = K^T.T @ Q^T
tiles land in PSUM, one ACT Exp instruction (scale=1/8 fused) moves them to
SBUF, and the V-matmuls accumulate attn@V over k-tiles. Normalization is a
DVE reciprocal + rank-1 ones-outer-product broadcast.

All matmul stationary operands are bf16 (K^T tiles, x tiles, weight chunks,
O^T tiles) with 128 columns where possible so the compiler's automatic Fast
Weight Load halves LDWEIGHTS; moving operands stay f32r (full-rate fp32).
v4/ot/kmod double-buffering lets iteration i+1's DMA + projections overlap
iteration i's attention tail under the For_i hardware loop.
"""

import numpy as np

B, N, DIM = 4, 2048, 512
HEADS, DH = 8, 64
N_CORES = 8
SCALE = DH ** -0.5  # 0.125

_RUNNER_CACHE = {}


# --------------------------------------------------------------------------
# Bass module
# --------------------------------------------------------------------------

def _build_nc(loop_n: int = 1):
    import concourse.mybir as mybir
    from concourse import bacc
    from concourse.tile import TileContext
    from concourse.masks import make_identity

    fp32 = mybir.dt.float32
    f32r = mybir.dt.float32r  # fp32 data at full matmul rate (producers round)
    bf16 = mybir.dt.bfloat16
    Exp = mybir.ActivationFunctionType.Exp

    nc = bacc.Bacc("TRN2", target_bir_lowering=False, debug=False)

    xT = nc.dram_tensor("xT", [DIM, N], bf16, kind="ExternalInput")
    yT = nc.dram_tensor("yT", [DIM, N], bf16, kind="ExternalInput")
    wq_d = nc.dram_tensor("wq", [DIM, 256], bf16, kind="ExternalInput")
    wk_d = nc.dram_tensor("wk", [DIM, 256], bf16, kind="ExternalInput")
    wv_d = nc.dram_tensor("wv", [DIM, 256], bf16, kind="ExternalInput")
    wfc_d = nc.dram_tensor("wfc", [DIM, DH], bf16, kind="ExternalInput")
    bfc_d = nc.dram_tensor("bfc", [DH, 1], fp32, kind="ExternalInput")
    wo_d = nc.dram_tensor("wo", [256, DIM], bf16, kind="ExternalInput")
    out_d = nc.dram_tensor("out", [N, DIM], fp32, kind="ExternalOutput")

    NT = N // 128   # 16 n-tiles of 128
    NS = N // 512   # 4  n-slices of 512
    DT = DIM // 128  # 4 contraction tiles

    with TileContext(nc) as tc:
        import contextlib
        with contextlib.ExitStack() as ctx:
            const = ctx.enter_context(tc.tile_pool(name="const", bufs=1))
            xtp = ctx.enter_context(tc.tile_pool(name="xtp", bufs=DT))
            ytp = ctx.enter_context(tc.tile_pool(name="ytp", bufs=4))
            modp = ctx.enter_context(tc.tile_pool(name="modp", bufs=1))
            v4p = ctx.enter_context(tc.tile_pool(name="v4p", bufs=2))
            qkp = ctx.enter_context(tc.tile_pool(name="qkp", bufs=2))
            otp = ctx.enter_context(tc.tile_pool(name="otp", bufs=4))
            ep = ctx.enter_context(tc.tile_pool(name="ep", bufs=4))
            accsp = ctx.enter_context(tc.tile_pool(name="accsp", bufs=4))
            rp = ctx.enter_context(tc.tile_pool(name="rp", bufs=4))
            outp = ctx.enter_context(tc.tile_pool(name="outp", bufs=4))
            # PSUM: mixps 2 banks + sps 2x[128,1024]=4 banks + accps 2 = 8
            mixps = ctx.enter_context(
                tc.tile_pool(name="mixps", bufs=2, space="PSUM"))
            sps = ctx.enter_context(
                tc.tile_pool(name="sps", bufs=2, space="PSUM"))
            accps = ctx.enter_context(
                tc.tile_pool(name="accps", bufs=2, space="PSUM"))

            def body(_i=None):
                # ---- constants / weights -------------------------------
                wq = const.tile([128, DT, 256], bf16, tag="wq")
                nc.sync.dma_start(wq[:, :, :],
                                  wq_d.rearrange("(t p) f -> p t f", p=128))
                wk = const.tile([128, DT, 256], bf16, tag="wk")
                nc.sync.dma_start(wk[:, :, :],
                                  wk_d.rearrange("(t p) f -> p t f", p=128))
                wv = const.tile([128, DT, 256], bf16, tag="wv")
                nc.sync.dma_start(wv[:, :, :],
                                  wv_d.rearrange("(t p) f -> p t f", p=128))
                # W_fc duplicated along free dim: one matmul yields the
                # modulation row-block for both heads of a pair.
                wfc2 = const.tile([128, DT, 128], bf16, tag="wfc2")
                wfc_r = wfc_d.rearrange("(t p) f -> p t f", p=128)
                nc.sync.dma_start(wfc2[:, :, 0:DH], wfc_r)
                nc.sync.dma_start(wfc2[:, :, DH:128], wfc_r)
                bfc2 = const.tile([128, 1], fp32, tag="bfc2")
                nc.sync.dma_start(bfc2[0:DH, :], bfc_d[:, :])
                nc.sync.dma_start(bfc2[DH:128, :], bfc_d[:, :])
                wo = const.tile([128, 2, DIM], bf16, tag="wo")
                nc.sync.dma_start(wo[:, :, :],
                                  wo_d.rearrange("(t p) f -> p t f", p=128))
                ident = const.tile([128, 128], fp32, tag="ident")
                make_identity(nc, ident[:, :])
                ones1 = const.tile([128, 1], fp32, tag="ones1")
                nc.gpsimd.memset(ones1[:, :], 1.0)
                ones_row = const.tile([1, DH], f32r, tag="ones_row")
                nc.vector.tensor_copy(ones_row[:, :],
                                      ones1[0:1, :].broadcast_to((1, DH)))

                # ---- okv^T (both heads' copy on 128 rows) --------------
                # y streamed per 512-slice; okv = fc(y) duplicated rows.
                okvT2 = modp.tile([128, N], fp32, tag="okvT2")
                for ns in range(NS):
                    sl = slice(ns * 512, (ns + 1) * 512)
                    ps = mixps.tile([128, 512], fp32, tag="ps")
                    for t in range(DT):
                        ytile = ytp.tile([128, 512], bf16, tag="yt")
                        nc.sync.dma_start(ytile[:, :],
                                          yT[t * 128:(t + 1) * 128, sl])
                        nc.tensor.matmul(ps[:, :], wfc2[:, t, :],
                                         ytile[:, :],
                                         start=(t == 0), stop=(t == DT - 1))
                    nc.vector.tensor_scalar_add(okvT2[:, sl], ps[:, :],
                                                bfc2[:, :])

                # x tiles (bf16, used as both moving and stationary operand)
                xt = []
                for t in range(DT):
                    xtile = xtp.tile([128, N], bf16, tag="xt")
                    nc.sync.dma_start(xtile[:, :],
                                      xT[t * 128:(t + 1) * 128, :])
                    xt.append(xtile)

                # okv in natural [N, dh] layout (for V modulation): PE
                # transpose of okv^T 128-column blocks.
                okvn = modp.tile([128, NT, DH], fp32, tag="okvn")
                for g in range(NT // 8):
                    tps = mixps.tile([128, 512], fp32, tag="ps")
                    for j in range(8):
                        nt = g * 8 + j
                        nc.tensor.transpose(
                            tps[:, j * DH:(j + 1) * DH],
                            okvT2[0:DH, nt * 128:(nt + 1) * 128],
                            ident[0:DH, 0:DH])
                    nc.vector.tensor_copy(okvn[:, g * 8:(g + 1) * 8, :],
                                          tps[:, :].rearrange(
                                              "p (n c) -> p n c", n=8))

                # ---- oq^T --------------------------------------------
                oqT2 = modp.tile([128, N], fp32, tag="oqT2")
                for ns in range(NS):
                    sl = slice(ns * 512, (ns + 1) * 512)
                    ps = mixps.tile([128, 512], fp32, tag="ps")
                    for t in range(DT):
                        nc.tensor.matmul(ps[:, :], wfc2[:, t, :],
                                         xt[t][:, sl],
                                         start=(t == 0), stop=(t == DT - 1))
                    nc.vector.tensor_scalar_add(oqT2[:, sl], ps[:, :],
                                                bfc2[:, :])

                # ---- V projection for all 4 heads (natural layout,
                # ones column per head for the softmax denominator) ------
                v4 = v4p.tile([128, NT, 260], bf16, tag="v4")
                ones_b = ones1[:, :].unsqueeze(1).broadcast_to((128, NT, 1))
                v4h = v4[:, :, :].rearrange("p n (h c) -> p n h c", h=4)
                nc.vector.tensor_copy(v4h[:, :, :, DH:DH + 1],
                                      ones_b.unsqueeze(2).broadcast_to(
                                          (128, NT, 4, 1)))
                for nt in range(0, NT, 2):
                    psv = mixps.tile([128, 512], fp32, tag="ps")
                    for half in range(2):
                        for t in range(DT):
                            nc.tensor.matmul(
                                psv[:, half * 256:half * 256 + 256],
                                xt[t][:, (nt + half) * 128:
                                       (nt + half + 1) * 128],
                                wv[:, t, :],
                                start=(t == 0), stop=(t == DT - 1))
                    okb = okvn[:, nt:nt + 2, :].unsqueeze(2).broadcast_to(
                        (128, 2, 4, DH))
                    nc.vector.tensor_mul(
                        v4[:, nt:nt + 2, :].rearrange(
                            "p n (h c) -> p n h c", h=4)[:, :, :, 0:DH],
                        psv[:, :].rearrange("p (n h c) -> p n h c", n=2, h=4),
                        okb)

                def q_proj(p, ns_list, qmod):
                    pf = slice(p * 128, (p + 1) * 128)
                    for ns in ns_list:
                        sl = slice(ns * 512, (ns + 1) * 512)
                        psq = mixps.tile([128, 512], fp32, tag="ps")
                        for t in range(DT):
                            nc.tensor.matmul(psq[:, :], wq[:, t, pf],
                                             xt[t][:, sl],
                                             start=(t == 0),
                                             stop=(t == DT - 1))
                        nc.vector.tensor_mul(qmod[:, sl], psq[:, :],
                                             oqT2[:, sl])

                def k_proj(p, ns_list, kmod):
                    pf = slice(p * 128, (p + 1) * 128)
                    for ns in ns_list:
                        sl = slice(ns * 512, (ns + 1) * 512)
                        psk = mixps.tile([128, 512], fp32, tag="ps")
                        for t in range(DT):
                            nc.tensor.matmul(psk[:, :], wk[:, t, pf],
                                             xt[t][:, sl],
                                             start=(t == 0),
                                             stop=(t == DT - 1))
                        nc.vector.tensor_mul(kmod[:, sl], psk[:, :],
                                             okvT2[:, sl])

                def attn_qt(p, qt, qmod, kmod, ot):
                    qsl = slice(qt * 512, (qt + 1) * 512)
                    acc0 = accps.tile([65, 512], fp32, tag="acc")
                    acc1 = accps.tile([65, 512], fp32, tag="acc")
                    for kt in range(NT):
                        ksl = slice(kt * 128, (kt + 1) * 128)
                        sp = sps.tile([128, 1024], fp32, tag="s")
                        nc.tensor.matmul(sp[:, 0:512],
                                         kmod[0:DH, ksl],
                                         qmod[0:DH, qsl],
                                         start=True, stop=True)
                        nc.tensor.matmul(sp[:, 512:1024],
                                         kmod[DH:128, ksl],
                                         qmod[DH:128, qsl],
                                         start=True, stop=True)
                        e = ep.tile([128, 1024], bf16, tag="e")
                        nc.scalar.activation(e[:, :], sp[:, :], Exp,
                                             scale=float(SCALE))
                        nc.tensor.matmul(acc0[:, :],
                                         v4[:, kt, p * 130:p * 130 + 65],
                                         e[:, 0:512],
                                         start=(kt == 0),
                                         stop=(kt == NT - 1))
                        nc.tensor.matmul(acc1[:, :],
                                         v4[:, kt, p * 130 + 65:p * 130 + 130],
                                         e[:, 512:1024],
                                         start=(kt == 0),
                                         stop=(kt == NT - 1))
                    for h, acc in ((0, acc0), (1, acc1)):
                        accS = accsp.tile([65, 512], fp32, tag="accS")
                        nc.vector.tensor_copy(accS[:, :], acc[:, :])
                        rec = rp.tile([1, 512], f32r, tag="rec")
                        with nc.allow_low_precision(
                                reason="f32r-typed fp32 reciprocal row"):
                            nc.vector.reciprocal(rec[:, :], accS[64:65, :])
                        bc = mixps.tile([128, 512], fp32, tag="ps")
                        nc.tensor.matmul(bc[0:DH, :], ones_row[:, :],
                                         rec[:, :], start=True, stop=True)
                        nc.vector.tensor_mul(ot[h * DH:(h + 1) * DH, qsl],
                                             accS[0:DH, :], bc[0:DH, :])

                # K^T tiles are matmul stationaries: bf16 + 128 cols -> FWL.
                kmod0 = qkp.tile([128, N], bf16, tag="kmod")
                k_proj(0, range(NS), kmod0)
                qmod0 = qkp.tile([128, N], bf16, tag="qmod")
                q_proj(0, [0], qmod0)
                kmod1 = qkp.tile([128, N], bf16, tag="kmod")
                qmod1 = qkp.tile([128, N], bf16, tag="qmod")
                ot0 = otp.tile([128, N], bf16, tag="ot")
                ot1 = otp.tile([128, N], bf16, tag="ot")
                ots = [ot0, ot1]
                # pair-0 attention, with the rest of the pair-0 Q projection
                # and the pair-1 QK projection interleaved into the PE idle
                # gaps of the ACT-bound attention loop.
                for qt in range(NS):
                    attn_qt(0, qt, qmod0, kmod0, ot0)
                    if qt < NS - 1:
                        q_proj(0, [qt + 1], qmod0)
                    if qt < 2:
                        k_proj(1, [2 * qt, 2 * qt + 1], kmod1)
                    else:
                        q_proj(1, [2 * (qt - 2), 2 * (qt - 2) + 1], qmod1)

                def outproj_nt(nt):
                    nsl = slice(nt * 128, (nt + 1) * 128)
                    pso = mixps.tile([128, 512], fp32, tag="ps")
                    nc.tensor.matmul(pso[:, :], ots[0][:, nsl],
                                     wo[:, 0, :], start=True, stop=False)
                    nc.tensor.matmul(pso[:, :], ots[1][:, nsl],
                                     wo[:, 1, :], start=False, stop=True)
                    ob = outp.tile([128, 512], fp32, tag="ob")
                    nc.vector.tensor_copy(ob[:, :], pso[:, :])
                    nc.sync.dma_start(out_d[nsl, :], ob[:, :])

                for qt in range(NS):
                    attn_qt(1, qt, qmod1, kmod1, ot1)
                    for nt in range(4 * qt, 4 * qt + 4):
                        outproj_nt(nt)

            if loop_n > 1:
                with tc.For_i(0, loop_n, 1) as _i:
                    body(_i)
            else:
                body()

    nc.compile()
    return nc


# --------------------------------------------------------------------------
# PJRT SPMD runner (axon path) — keeps the jitted callable for reuse
# --------------------------------------------------------------------------

class _SpmdRunner:
    def __init__(self, nc, n_cores):
        import jax
        from jax.sharding import Mesh, PartitionSpec, NamedSharding
        from jax.experimental.shard_map import shard_map
        import concourse.mybir as mybir
        from concourse import bass2jax
        from concourse.bass2jax import _bass_exec_p, install_neuronx_cc_hook

        install_neuronx_cc_hook()
        self.jax = jax
        self.nc = nc
        self.n_cores = n_cores
        pname = nc.partition_id_tensor.name if nc.partition_id_tensor else None
        in_names, out_names, out_avals, zero_shapes = [], [], [], []
        for alloc in nc.m.functions[0].allocations:
            if not isinstance(alloc, mybir.MemoryLocationSet):
                continue
            name = alloc.memorylocations[0].name
            if alloc.kind == "ExternalInput":
                if name != pname:
                    in_names.append(name)
            elif alloc.kind == "ExternalOutput":
                out_names.append(name)
                shape = tuple(alloc.tensor_shape)
                dtype = mybir.dt.np(alloc.dtype)
                out_avals.append(jax.core.ShapedArray(shape, dtype))
                zero_shapes.append((shape, dtype))
        self.n_params = len(in_names)
        self.in_names = list(in_names)
        self.out_names = out_names
        self.out_avals = out_avals
        all_names = in_names + out_names
        if pname is not None:
            all_names.append(pname)

        def _body(*args):
            operands = list(args)
            if pname is not None:
                operands.append(bass2jax.partition_id_tensor())
            return tuple(_bass_exec_p.bind(
                *operands, out_avals=tuple(out_avals),
                in_names=tuple(all_names), out_names=tuple(out_names),
                lowering_input_output_aliases=(),
                sim_require_finite=True, sim_require_nnan=True, nc=nc))

        devices = jax.devices()[:n_cores]
        self.mesh = Mesh(np.asarray(devices), ("core",))
        n_outs = len(out_avals)
        in_specs = (PartitionSpec("core"),) * (self.n_params + n_outs)
        out_specs = (PartitionSpec("core"),) * n_outs
        donate = tuple(range(self.n_params, self.n_params + n_outs))
        self.sharding = NamedSharding(self.mesh, PartitionSpec("core"))
        self.sharded = jax.jit(
            shard_map(_body, mesh=self.mesh, in_specs=in_specs,
                      out_specs=out_specs, check_rep=False),
            donate_argnums=donate, keep_unused=True)
        zs = [(n_cores * s[0], *s[1:]) for s, _ in zero_shapes]
        zd = [d for _, d in zero_shapes]
        self._mkzeros = jax.jit(
            lambda: tuple(jax.numpy.zeros(s, d) for s, d in zip(zs, zd)),
            out_shardings=tuple(self.sharding for _ in zs))

    def put_inputs(self, in_maps):
        concat = [np.concatenate(
            [np.ascontiguousarray(in_maps[c][n]) for c in range(self.n_cores)],
            axis=0) for n in self.in_names]
        return [self.jax.device_put(a, self.sharding) for a in concat]

    def run(self, in_dev):
        outs = self.sharded(*in_dev, *self._mkzeros())
        self.jax.block_until_ready(outs)
        return outs

    def results(self, outs):
        res = []
        for c in range(self.n_cores):
            d = {}
            for i, name in enumerate(self.out_names):
                full = np.asarray(outs[i])
                d[name] = full.reshape(self.n_cores,
                                       *self.out_avals[i].shape)[c]
            res.append(d)
        return res


def _get_runner(loop_n: int = 1):
    if loop_n not in _RUNNER_CACHE:
        nc = _build_nc(loop_n)
        _RUNNER_CACHE[loop_n] = _SpmdRunner(nc, N_CORES)
    return _RUNNER_CACHE[loop_n]


# --------------------------------------------------------------------------
# host-side shard / gather
# --------------------------------------------------------------------------

def _shard_inputs(x, y, W_qkv, W_fc, b_fc, W_out):
    import ml_dtypes
    bf = ml_dtypes.bfloat16
    x = np.asarray(x)
    y = np.asarray(y)
    W_qkv = np.asarray(W_qkv)
    W_fc = np.asarray(W_fc, dtype=np.float32)
    b_fc = np.asarray(b_fc, dtype=np.float32)
    W_out = np.asarray(W_out)
    xT = [np.ascontiguousarray(x[b].T).astype(bf) for b in range(B)]
    yT = [np.ascontiguousarray(y[b].T).astype(bf) for b in range(B)]
    wq_g = [np.ascontiguousarray(W_qkv[:, g * 256:(g + 1) * 256]).astype(bf)
            for g in range(2)]
    wk_g = [np.ascontiguousarray(
                W_qkv[:, 512 + g * 256:512 + (g + 1) * 256]).astype(bf)
            for g in range(2)]
    wv_g = [np.ascontiguousarray(
                W_qkv[:, 1024 + g * 256:1024 + (g + 1) * 256]).astype(bf)
            for g in range(2)]
    wfc = np.ascontiguousarray(W_fc).astype(bf)
    bfc = np.ascontiguousarray(b_fc.reshape(DH, 1))
    wo_g = [np.ascontiguousarray(W_out[g * 256:(g + 1) * 256, :]).astype(bf)
            for g in range(2)]
    in_maps = []
    for c in range(N_CORES):
        b, g = c // 2, c % 2
        in_maps.append({
            "xT": xT[b], "yT": yT[b],
            "wq": wq_g[g], "wk": wk_g[g], "wv": wv_g[g],
            "wfc": wfc, "bfc": bfc, "wo": wo_g[g],
        })
    return in_maps


def kernel(x, y, W_qkv, W_fc, b_fc, W_out, b_out):
    runner = _get_runner(1)
    in_maps = _shard_inputs(x, y, W_qkv, W_fc, b_fc, W_out)
    in_dev = runner.put_inputs(in_maps)
    res = runner.results(runner.run(in_dev))
    b_out = np.asarray(b_out, dtype=np.float32)
    out = np.empty((B, N, DIM), dtype=np.float32)
    for b in range(B):
        out[b] = res[2 * b]["out"] + res[2 * b + 1]["out"] + b_out
    return out
